# revision 6
# baseline (speedup 1.0000x reference)
"""Trainium2 Bass kernel for nn_HRNetW30classifier: logits = x @ W.T + b.

Shapes (full): x (8192, 2048) f32, W (1000, 2048) f32, b (1000,) f32
Output: (8192, 1000) f32.

Sharding: data-parallel over batch across 8 NeuronCores. Each core computes a
(1024, 2048) @ (2048, 1000) GEMM with W/b replicated.

Device kernel layout: host pre-transposes x and W so the contraction dim (K=2048
features) lands on the SBUF partition axis, giving fully-contiguous DMA rows.
The per-core GEMM runs on the TensorEngine in float32r mode (full fp32 input
bytes, ~1 cycle/row when the moving free dim >= 256), accumulating in PSUM fp32
over 16 K-tiles, with the bias add fused into the PSUM->SBUF eviction.
"""

import numpy as np

P = 128
N_CORES = 8
B_FULL = 8192
M = B_FULL // N_CORES  # 1024 batch rows per core
N = 1000  # classes
K = 2048  # features
KT = K // P  # 16 k-tiles
MT = M // P  # 8 m-tiles
N0_W = 512  # first n-chunk (one PSUM bank of fp32)
N1_W = N - N0_W  # 488

_NC_CACHE = {}


def _build_nc():
    """Build + compile the per-core Bass program (SPMD: same NEFF on 8 cores)."""
    from contextlib import ExitStack

    import concourse.tile as tile
    from concourse import bacc, mybir
    from concourse._compat import get_trn_type

    f32 = mybir.dt.float32
    f32r = mybir.dt.float32r

    nc = bacc.Bacc(get_trn_type() or "TRN2", target_bir_lowering=False, debug=False)

    # float32r (= TF32) end-to-end for the matmul operands: the BIR verifier
    # requires fp32r matmul inputs to come from an fp32r-rounded producer
    # chain, so the host pre-rounds and the DMA just moves rounded bits.
    xT = nc.dram_tensor("xT", [K, M], f32r, kind="ExternalInput")
    wT = nc.dram_tensor("wT", [K, N], f32r, kind="ExternalInput")
    bias = nc.dram_tensor("bias", [P, N], f32, kind="ExternalInput")
    out = nc.dram_tensor("out", [M, N], f32, kind="ExternalOutput")

    xT_r = xT.ap().rearrange("(kt p) m -> kt p m", p=P)  # [KT, 128, M]
    wT_r = wT.ap().rearrange("(kt p) n -> kt p n", p=P)  # [KT, 128, N]
    out_r = out.ap().rearrange("(mt p) n -> mt p n", p=P)  # [MT, 128, N]

    with tile.TileContext(nc) as tc:
        with ExitStack() as ctx:
            xpool = ctx.enter_context(tc.tile_pool(name="xpool", bufs=1))
            w0pool = ctx.enter_context(tc.tile_pool(name="w0pool", bufs=1))
            w1pool = ctx.enter_context(tc.tile_pool(name="w1pool", bufs=1))
            bpool = ctx.enter_context(tc.tile_pool(name="bpool", bufs=1))
            opool = ctx.enter_context(tc.tile_pool(name="opool", bufs=4))
            pspool = ctx.enter_context(tc.tile_pool(name="ps", bufs=8, space="PSUM"))

            bias_t = bpool.tile([P, N], f32, tag="bias")
            nc.sync.dma_start(bias_t[:], bias.ap())

            # Everything is resident in SBUF: x (64KB/part), W (62.5KB/part).
            x_sb = xpool.tile([P, KT, M], f32r, tag="x")
            w0_sb = w0pool.tile([P, KT, N0_W], f32r, tag="w0")
            w1_sb = w1pool.tile([P, KT, N1_W], f32r, tag="w1")

            # Phase-A loads (needed first): x + first 512 classes of W.
            for kt in range(KT):
                nc.sync.dma_start(w0_sb[:, kt, :], wT_r[kt][:, 0:N0_W])
                nc.sync.dma_start(x_sb[:, kt, :], xT_r[kt])
            # Phase-B loads: remaining 488 classes, only needed ~27us later.
            for kt in range(KT):
                nc.sync.dma_start(w1_sb[:, kt, :], wT_r[kt][:, N0_W:N])

            for n_idx, (n0, nw, w_sb) in enumerate(
                ((0, N0_W, w0_sb), (N0_W, N1_W, w1_sb))
            ):
                ps_tiles = [
                    pspool.tile([P, N0_W], f32, tag="ps", name=f"ps_{n_idx}_{mt}")
                    for mt in range(MT)
                ]
                for kt in range(KT):
                    for mt in range(MT):
                        nc.tensor.matmul(
                            ps_tiles[mt][:, :nw],
                            lhsT=x_sb[:, kt, mt * P : (mt + 1) * P],
                            rhs=w_sb[:, kt, :],
                            start=(kt == 0),
                            stop=(kt == KT - 1),
                        )
                for mt in range(MT):
                    ot = opool.tile([P, N0_W], f32, tag="ot")
                    nc.vector.tensor_add(
                        ot[:, :nw], ps_tiles[mt][:, :nw], bias_t[:, n0 : n0 + nw]
                    )
                    nc.sync.dma_start(out_r[mt, :, n0 : n0 + nw], ot[:, :nw])

    nc.compile()
    return nc


def _get_nc():
    if "nc" not in _NC_CACHE:
        _NC_CACHE["nc"] = _build_nc()
    return _NC_CACHE["nc"]


def _run(in_maps, trace=False, **kwargs):
    from concourse.bass_utils import run_bass_kernel_spmd

    nc = _get_nc()
    return run_bass_kernel_spmd(
        nc, in_maps, core_ids=list(range(N_CORES)), trace=trace, **kwargs
    )


def _round_tf32(a):
    """Round fp32 to the fp32r/TF32 grid (10 mantissa bits, RNE) like
    walrus's cast_fp32_to_fp32r expects of fp32r matmul inputs."""
    u = np.ascontiguousarray(a, dtype=np.float32).view(np.uint32)
    r = u + 0x00000FFF + ((u >> 13) & 1)
    return (r & np.uint32(0xFFFFE000)).view(np.float32)


def _make_in_maps(x, W, b):
    x = np.asarray(x, dtype=np.float32)
    W = np.asarray(W, dtype=np.float32)
    b = np.asarray(b, dtype=np.float32)
    xT = _round_tf32(np.ascontiguousarray(x.T))  # (K, B_FULL)
    wT = _round_tf32(np.ascontiguousarray(W.T))  # (K, N)
    bias = np.ascontiguousarray(np.broadcast_to(b[None, :], (P, N)))
    return [
        {
            "xT": np.ascontiguousarray(xT[:, c * M : (c + 1) * M]),
            "wT": wT,
            "bias": bias,
        }
        for c in range(N_CORES)
    ]


def kernel(x, W, b):
    res = _run(_make_in_maps(x, W, b))
    return np.concatenate([r["out"] for r in res.results], axis=0)


# revision 7
# speedup vs baseline: 1.0204x; 1.0204x over previous
"""Trainium2 Bass kernel for nn_HRNetW30classifier: logits = x @ W.T + b.

Shapes (full): x (8192, 2048) f32, W (1000, 2048) f32, b (1000,) f32
Output: (8192, 1000) f32.

Sharding: data-parallel over batch across 8 NeuronCores. Each core computes a
(1024, 2048) @ (2048, 1000) GEMM with W/b replicated.

Device kernel: host pre-transposes x and W so the contraction dim (K=2048)
lands on the SBUF partition axis (contiguous DMA rows) and pre-rounds both to
the fp32r/TF32 grid. The TensorEngine runs float32r matmuls (~4x the fp32
rate), accumulating fp32 in PSUM over 16 K-tiles; bias is added during the
PSUM->SBUF eviction on the VectorEngine.

Schedule: N=1000 is split into column chunks (512, 488) so each accumulation
group fits one PSUM bank; 8 groups (one per 128-row m-tile) are live at a
time. The n0 pass is K-outer so the PE consumes each (w0[k], x[k]) DMA pair
right as it lands (~358 GB/s sustained). Input DMAs are chained with a
4-deep dependency window so they complete in need-order instead of
round-robin racing across queues. The n1 pass is K-outer while w1 streams,
then switches to group-serial for the last K-tiles so the 8 final evictions
stagger instead of piling up after the last matmul.
"""

import numpy as np

P = 128
N_CORES = 8
B_FULL = 8192
M = B_FULL // N_CORES  # 1024 batch rows per core
N = 1000  # classes
K = 2048  # features
KT = K // P  # 16 k-tiles
MT = M // P  # 8 m-tiles
N0_W = 512  # first n-chunk (one PSUM bank of fp32)
N1_W = N - N0_W  # 488
KT_SPLIT = 8  # n1 pass: k-outer for kt<KT_SPLIT, group-serial after
DMA_WINDOW = 4  # in-flight input DMA window (completion ~= need order)

_NC_CACHE = {}


def _build_nc():
    """Build + compile the per-core Bass program (SPMD: same NEFF on 8 cores)."""
    from contextlib import ExitStack

    import concourse.tile as tile
    from concourse import bacc, mybir
    from concourse._compat import get_trn_type
    from concourse.tile_rust import add_dep_helper

    f32 = mybir.dt.float32
    f32r = mybir.dt.float32r

    nc = bacc.Bacc(get_trn_type() or "TRN2", target_bir_lowering=False, debug=False)

    xT = nc.dram_tensor("xT", [K, M], f32r, kind="ExternalInput")
    wT = nc.dram_tensor("wT", [K, N], f32r, kind="ExternalInput")
    bias = nc.dram_tensor("bias", [P, N], f32, kind="ExternalInput")
    out = nc.dram_tensor("out", [M, N], f32, kind="ExternalOutput")

    xT_r = xT.ap().rearrange("(kt p) m -> kt p m", p=P)  # [KT, 128, M]
    wT_r = wT.ap().rearrange("(kt p) n -> kt p n", p=P)  # [KT, 128, N]
    out_r = out.ap().rearrange("(mt p) n -> mt p n", p=P)  # [MT, 128, N]

    with tile.TileContext(nc) as tc:
        with ExitStack() as ctx:
            xpool = ctx.enter_context(tc.tile_pool(name="xpool", bufs=1))
            w0pool = ctx.enter_context(tc.tile_pool(name="w0pool", bufs=1))
            w1pool = ctx.enter_context(tc.tile_pool(name="w1pool", bufs=1))
            bpool = ctx.enter_context(tc.tile_pool(name="bpool", bufs=1))
            opool = ctx.enter_context(tc.tile_pool(name="opool", bufs=8))
            pspool = ctx.enter_context(tc.tile_pool(name="ps", bufs=8, space="PSUM"))

            bias_t = bpool.tile([P, N], f32, tag="bias")
            nc.sync.dma_start(bias_t[:], bias.ap())

            # Everything is resident in SBUF: x (64KB/part), W (62.5KB/part).
            x_sb = xpool.tile([P, KT, M], f32r, tag="x")
            w0_sb = w0pool.tile([P, KT, N0_W], f32r, tag="w0")
            w1_sb = w1pool.tile([P, KT, N1_W], f32r, tag="w1")

            # Input DMA stream in need-order, chained with a sliding window so
            # completion order tracks issue order (otherwise all DMAs race
            # round-robin across queues and the first k-slice lands late).
            chain = []
            for kt in range(KT):
                chain.append(nc.sync.dma_start(w0_sb[:, kt, :], wT_r[kt][:, 0:N0_W]))
                chain.append(nc.sync.dma_start(x_sb[:, kt, :], xT_r[kt]))
            for kt in range(KT):
                chain.append(nc.sync.dma_start(w1_sb[:, kt, :], wT_r[kt][:, N0_W:N]))
            for i in range(DMA_WINDOW, len(chain)):
                add_dep_helper(
                    chain[i].ins,
                    chain[i - DMA_WINDOW].ins,
                    True,
                    "input stream need-order window",
                )

            def evict(ps_t, mt, n0, nw):
                ot = opool.tile([P, N0_W], f32, tag="ot", name=f"ot_{n0}_{mt}")
                nc.vector.tensor_add(ot[:, :nw], ps_t[:, :nw], bias_t[:, n0 : n0 + nw])
                nc.sync.dma_start(out_r[mt, :, n0 : n0 + nw], ot[:, :nw])

            # ---- n0 pass: k-outer, paced by the (w0[k], x[k]) DMA stream ----
            ps0 = [
                pspool.tile([P, N0_W], f32, tag="ps", name=f"ps0_{mt}")
                for mt in range(MT)
            ]
            for kt in range(KT):
                for mt in range(MT):
                    nc.tensor.matmul(
                        ps0[mt][:, :N0_W],
                        lhsT=x_sb[:, kt, mt * P : (mt + 1) * P],
                        rhs=w0_sb[:, kt, :],
                        start=(kt == 0),
                        stop=(kt == KT - 1),
                    )
            for mt in range(MT):
                evict(ps0[mt], mt, 0, N0_W)

            # ---- n1 pass: k-outer while w1 streams in ----
            ps1 = [
                pspool.tile([P, N0_W], f32, tag="ps", name=f"ps1_{mt}")
                for mt in range(MT)
            ]
            for kt in range(KT_SPLIT):
                for mt in range(MT):
                    nc.tensor.matmul(
                        ps1[mt][:, :N1_W],
                        lhsT=x_sb[:, kt, mt * P : (mt + 1) * P],
                        rhs=w1_sb[:, kt, :],
                        start=(kt == 0),
                        stop=False,
                    )
            # ---- n1 tail: group-serial so evictions stagger ----
            for mt in range(MT):
                for kt in range(KT_SPLIT, KT):
                    nc.tensor.matmul(
                        ps1[mt][:, :N1_W],
                        lhsT=x_sb[:, kt, mt * P : (mt + 1) * P],
                        rhs=w1_sb[:, kt, :],
                        start=False,
                        stop=(kt == KT - 1),
                    )
                evict(ps1[mt], mt, N0_W, N1_W)

    nc.compile()
    return nc


def _get_nc():
    if "nc" not in _NC_CACHE:
        _NC_CACHE["nc"] = _build_nc()
    return _NC_CACHE["nc"]


def _run(in_maps, trace=False, **kwargs):
    from concourse.bass_utils import run_bass_kernel_spmd

    nc = _get_nc()
    return run_bass_kernel_spmd(
        nc, in_maps, core_ids=list(range(N_CORES)), trace=trace, **kwargs
    )


def _round_tf32(a):
    """Round fp32 to the fp32r/TF32 grid (10 mantissa bits, RNE) like
    walrus's cast_fp32_to_fp32r expects of fp32r matmul inputs."""
    u = np.ascontiguousarray(a, dtype=np.float32).view(np.uint32)
    r = u + 0x00000FFF + ((u >> 13) & 1)
    return (r & np.uint32(0xFFFFE000)).view(np.float32)


def _make_in_maps(x, W, b):
    x = np.asarray(x, dtype=np.float32)
    W = np.asarray(W, dtype=np.float32)
    b = np.asarray(b, dtype=np.float32)
    xT = _round_tf32(np.ascontiguousarray(x.T))  # (K, B_FULL)
    wT = _round_tf32(np.ascontiguousarray(W.T))  # (K, N)
    bias = np.ascontiguousarray(np.broadcast_to(b[None, :], (P, N)))
    return [
        {
            "xT": np.ascontiguousarray(xT[:, c * M : (c + 1) * M]),
            "wT": wT,
            "bias": bias,
        }
        for c in range(N_CORES)
    ]


def kernel(x, W, b):
    res = _run(_make_in_maps(x, W, b))
    return np.concatenate([r["out"] for r in res.results], axis=0)
